# revision 2
# baseline (speedup 1.0000x reference)
"""Trainium2 Bass kernel for the Dedicom decoder problem — v5.

Math: with U = z * d, scores S = A @ U^T (A = U @ W host-precomputed).
Sharding 2 row-shards x 4 col-shards: core (i, j) owns S rows
[i*2048, (i+1)*2048) x cols [j*1024, (j+1)*1024), processed as logical
units (row-tile, col-range) computed with fp8 DoubleRow matmuls
(a8T x SU^2, zT8 x SU host-packed) and cast PSUM->SBUF bf16 (split
ACT/DVE).  'p' units: GPSIMD indirect_copy pulls each edge's bf16 PAIR
(f32-bitcast scan) into slot tiles that are dump-DMAd to DRAM.  'd'
units: the whole bf16 block is dump-DMAd (no gather).  Unit 0 is split
into column halves so the cast chain starts earlier.  The host picks
the bf16 lane, rescales, applies sigmoid and unscatters; no mask, no
select, no segsum and no sigmoid run on device.
"""

import numpy as np
import ml_dtypes

BF = ml_dtypes.bfloat16
F8 = ml_dtypes.float8_e4m3fn

N_DRUGS = 4096
D = 512
N_CORES = 8
RSH, CSH = 2, 4           # row shards x col shards
RBLK = N_DRUGS // RSH     # 2048 rows of S per core
CBLK = N_DRUGS // CSH     # 1024 cols of S per core
NT = RBLK // 128          # 16 row-tile units per core
SU = 16.0                 # fp8 dynamic-range pre-scale
PKW = CBLK + RBLK         # packed matrix cols: zT8q | a8T

# logical units: (row_tile, col_lo, col_hi, kind)
# kind 'p' = pair gather + slot dump, 'd' = full bf16 block dump.
# Unit 0 split into halves pulls the first cast ~1us earlier.
LUNITS = tuple(
    (t, 0, 1024, "d" if t % 2 == 1 else "p") for t in range(NT))
NL = len(LUNITS)
ACT_CAST = {0, 2, 4, 6, 8, 10, 12, 14, 15}  # ACT; rest DVE
N_WARM = 10                             # PE p-state warmup matmuls
# dump groups (logical 'p' units); DMA fires after the last one
GROUPS = ((0, 2), (4, 6), (8, 10), (12, 14))
# dma split points for the pack input (cols of [zT8q | a8T]);
# "ix" marks where the index-pack DMA is issued
DMA_SPLITS = ((0, 512), (CBLK, CBLK + 512), (512, CBLK), "ix",
              (CBLK + 512, CBLK + 1024), (CBLK + 1024, PKW))
MEMSET_ENG = "vector"                   # engine for warmup memsets

_cache = {}


def _lp():
    return tuple(i for i, u in enumerate(LUNITS) if u[3] == "p")


def _ld():
    return tuple(i for i, u in enumerate(LUNITS) if u[3] == "d")


def _offsets(nvs):
    """ix / out1 / out4 offsets per logical unit."""
    ixoff = np.zeros(NL + 1, np.int64)
    for i, (t, c0, c1, kd) in enumerate(LUNITS):
        ixoff[i + 1] = ixoff[i] + (nvs[i] // 16 if kd == "p" else 0)
    ooff = np.zeros(NL, np.int64)
    acc = 0
    for i in _lp():
        ooff[i] = acc
        acc += nvs[i]
    dacc = 0
    for i in _ld():
        ooff[i] = dacc
        dacc += LUNITS[i][2] - LUNITS[i][1]
    return ixoff, ooff, acc, dacc


def _build(nvs):
    import concourse.bass as bass  # noqa: F401
    import concourse.bacc as bacc
    import concourse.mybir as mybir
    import concourse.tile as tile

    f32 = mybir.dt.float32
    bf16 = mybir.dt.bfloat16
    fp8 = mybir.dt.float8e4
    u16 = mybir.dt.uint16
    u32 = mybir.dt.uint32
    DR = mybir.MatmulPerfMode.DoubleRow

    ixoff, ooff, pw, dw = _offsets(nvs)

    nc = bacc.Bacc("TRN2", target_bir_lowering=False, debug=False,
                   num_devices=N_CORES)

    PK = nc.dram_tensor("pack", [D, PKW], fp8, kind="ExternalInput")
    IX = nc.dram_tensor("ix", [128, int(ixoff[-1])], u16,
                        kind="ExternalInput")
    OUT1 = nc.dram_tensor("out1", [128, max(1, int(pw))], u32,
                          kind="ExternalOutput")
    OUT4 = nc.dram_tensor("out4", [128, max(1, int(dw))], u16,
                          kind="ExternalOutput")

    with tile.TileContext(nc) as tc:
        with (
            tc.tile_pool(name="big", bufs=1) as big,
            tc.tile_pool(name="sml", bufs=1) as sml,
            tc.tile_pool(name="psum", bufs=8, space="PSUM") as psum,
        ):
            pk_sb = big.tile([128, 4, PKW], fp8)
            pk_v = PK.ap().rearrange("(kc p) n -> p kc n", p=128)
            for sp in DMA_SPLITS:
                if sp == "ix":
                    xg_sb = sml.tile([128, int(ixoff[-1])], u16)
                    nc.sync.dma_start(xg_sb[:], IX.ap())
                else:
                    c0, c1 = sp
                    nc.sync.dma_start(pk_sb[:, :, c0:c1], pk_v[:, :, c0:c1])

            # PE p-state warmup during dma0: early anchor matmul pins
            # pe_busy_start, then a short burst bridges to the real work
            mseng = getattr(nc, MEMSET_ENG)
            wu_a = sml.tile([128, 2, 128], fp8)
            mseng.memset(wu_a[:], 0.0)
            wu = sml.tile([128, 2, 512], fp8)
            aps = psum.tile([128, 128], f32, tag="ps", bufs=4, name="wu_a")
            nc.tensor.matmul(aps[:], wu_a[:], wu_a[:],
                             start=True, stop=True, perf_mode=DR)
            mseng.memset(wu[:], 0.0)
            for i in range(N_WARM):
                wps = psum.tile([128, 512], f32, tag="ps", bufs=4,
                                name=f"wu_{i}")
                nc.tensor.matmul(wps[:], wu[:, :, 0:128], wu[:],
                                 start=True, stop=True, perf_mode=DR)

            # logical unit: DR matmuls -> psum -> bf16 cast -> gather or
            # full dump; 'p' slot tiles dump per GROUPS
            gtile = {}
            for li, (t, c0, c1, kd) in enumerate(LUNITS):
                cw = c1 - c0
                ps = psum.tile([128, cw], f32, tag="ps", bufs=4,
                               name=f"s_{li}")
                for nch in range(cw // 512):
                    for jc2 in range(2):
                        nc.tensor.matmul(
                            ps[:, nch * 512:(nch + 1) * 512],
                            pk_sb[:, 2 * jc2:2 * jc2 + 2,
                                  CBLK + t * 128:CBLK + (t + 1) * 128],
                            pk_sb[:, 2 * jc2:2 * jc2 + 2,
                                  c0 + nch * 512:c0 + (nch + 1) * 512],
                            start=(jc2 == 0), stop=(jc2 == 1), perf_mode=DR)
                sw = big.tile([128, cw], bf16, name=f"sw_{li}", tag="sw",
                              bufs=17)
                if li in ACT_CAST:
                    nc.scalar.copy(sw[:], ps[:])
                else:
                    nc.vector.tensor_copy(sw[:], ps[:])
                if kd == "d":
                    off = int(ooff[li])
                    nc.sync.dma_start(OUT4.ap()[:, off:off + cw],
                                      sw[:].bitcast(u16))
                    continue
                ixv = xg_sb[:, int(ixoff[li]):int(ixoff[li + 1])]
                grp = next(gr for gr in GROUPS if li in gr)
                gi = grp.index(li)
                gw = sum(nvs[u] for u in grp)
                if gi == 0:
                    gtile[grp] = big.tile([128, gw], f32, name=f"g_{li}")
                gt = gtile[grp]
                goff = sum(nvs[u] for u in grp[:gi])
                nc.gpsimd.indirect_copy(gt[:, goff:goff + nvs[li]],
                                        sw[:].bitcast(f32),
                                        ixv,
                                        i_know_ap_gather_is_preferred=True)
                if gi == len(grp) - 1:
                    off = int(ooff[grp[0]])
                    nc.sync.dma_start(
                        OUT1.ap()[:, off:off + gw],
                        gt[:].bitcast(u32))

    nc.compile()
    return nc


def _get_program(nvs):
    if nvs not in _cache:
        _cache[nvs] = _build(nvs)
    return _cache[nvs]


def kernel(z_drug, global_weight, local_diag, batch_edges, edge_sub_type_idx,
           **_unused):
    from concourse.bass_utils import run_bass_kernel_spmd

    z = np.asarray(z_drug, np.float32)
    W = np.asarray(global_weight, np.float32)
    ld = np.asarray(local_diag, np.float32)
    e = np.asarray(batch_edges)
    sub = int(np.asarray(edge_sub_type_idx))
    d = ld[sub]
    assert z.shape == (N_DRUGS, D) and W.shape == (D, D)
    B = e.shape[1]
    e0 = e[0].astype(np.int64)
    e1 = e[1].astype(np.int64)

    U = z * d                                           # [4096, 512] f32
    A = U @ W                                           # [4096, 512] f32
    zT8 = np.ascontiguousarray((U.T * SU)).astype(F8)   # [512, 4096] fp8
    a8T = np.ascontiguousarray((A.T * (SU * SU))).astype(F8)

    # per-row-tile logical unit lookup: [t, col>=512?] -> logical unit
    lu_of = np.zeros((NT, 2), np.int64)
    for i, (t, c0, c1, kd) in enumerate(LUNITS):
        if c0 == 0:
            lu_of[t, 0] = i
        if c1 == CBLK:
            lu_of[t, 1] = i
    lu_c0 = np.array([u[1] for u in LUNITS])
    is_d = np.array([u[3] == "d" for u in LUNITS])

    core = (e0 // RBLK) * CSH + e1 // CBLK
    rin = e0 % RBLK
    t = rin >> 7                                        # row tile
    g = (rin >> 4) & 7                                  # 16-partition group
    part = rin & 127
    cq = e1 % CBLK                                      # col within quarter
    lu = lu_of[t, (cq >= 512).astype(np.int64)]
    crel = cq - lu_c0[lu]                               # col within unit
    idx = crel >> 1                                     # f32 pair index

    # slot within each (core, lunit, group) bucket; nv padded per lunit
    order = np.lexsort((np.arange(B), g, lu, core))
    key = ((core * NL + lu) * 8 + g)[order]
    nb = N_CORES * NL * 8
    start = np.searchsorted(key, np.arange(nb), side="left")
    counts = np.bincount(key, minlength=nb)
    slot = np.arange(B) - start[key]
    cmax = counts.reshape(N_CORES, NL, 8).max(axis=(0, 2))
    # nv multiple of 32 keeps every ix slice 4-byte aligned (ISA mem4d)
    nvs = tuple(max(32, int(-(-int(c) // 32)) * 32) for c in cmax)
    ixoff, ooff, pw, dw = _offsets(nvs)

    cs = core[order]
    in_maps = []
    recs = []
    for c in range(N_CORES):
        m = order[cs == c]
        luc, gc = lu[m], g[m]
        ic = slot[cs == c]
        pm0 = ~is_d[luc]
        ix = np.zeros((128, int(ixoff[-1])), np.uint16)
        ix[16 * gc[pm0] + ic[pm0] % 16,
           ixoff[luc[pm0]] + ic[pm0] // 16] = idx[m][pm0].astype(np.uint16)
        ci, cj = divmod(c, CSH)
        pack = np.concatenate(
            [zT8[:, cj * CBLK:(cj + 1) * CBLK],
             a8T[:, ci * RBLK:(ci + 1) * RBLK]], axis=1)  # [512, PKW] fp8
        in_maps.append({"pack": pack, "ix": ix})
        recs.append((m, part[m], luc, ic, crel[m]))

    nc = _get_program(nvs)
    res = run_bass_kernel_spmd(nc, in_maps, list(range(N_CORES)))

    inv = 1.0 / (SU * SU * SU)
    out = np.empty(B, np.float32)
    for c in range(N_CORES):
        o1 = np.asarray(res.results[c]["out1"])         # [128, :] u32
        o4 = np.asarray(res.results[c]["out4"])         # [128, :] u16
        w1 = o1.view(np.uint16).reshape(128, -1, 2)     # bf16 lanes
        m, pt, luc, ic, crl = recs[c]
        dm = is_d[luc]
        pm = ~dm
        u16v = np.empty(len(m), np.uint16)
        u16v[dm] = o4[pt[dm], ooff[luc[dm]] + crl[dm]]
        u16v[pm] = w1[pt[pm], ooff[luc[pm]] + ic[pm], crl[pm] & 1]
        sc = (u16v.astype(np.uint32) << 16).view(np.float32) * inv
        out[m] = 1.0 / (1.0 + np.exp(-sc))
    return out


if __name__ == "__main__":
    dat = np.load("/root/problem/cached_io.npz")
    inputs = {k: dat[k] for k in ("z_drug", "global_weight", "local_diag",
                                  "batch_edges", "edge_sub_type_idx")}
    expected = dat["expected"]
    actual = kernel(**inputs)
    err = np.abs(actual - expected)
    print("max abs err:", err.max(), "mean:", err.mean())
    print("Relative error:", err.max() / np.abs(expected).max())


# revision 3
# speedup vs baseline: 1.0146x; 1.0146x over previous
"""Trainium2 Bass kernel for the Dedicom decoder problem — v5.

Math: with U = z * d, scores S = A @ U^T (A = U @ W host-precomputed).
Sharding 2 row-shards x 4 col-shards: core (i, j) owns S rows
[i*2048, (i+1)*2048) x cols [j*1024, (j+1)*1024), processed as logical
units (row-tile, col-range) computed with fp8 DoubleRow matmuls
(a8T x SU^2, zT8 x SU host-packed) and cast PSUM->SBUF bf16 (split
ACT/DVE).  'p' units: GPSIMD indirect_copy pulls each edge's bf16 PAIR
(f32-bitcast scan) into slot tiles that are dump-DMAd to DRAM.  'd'
units: the whole bf16 block is dump-DMAd (no gather; the host indexes
it directly).  Interleaving p/d keeps the SP DMA queue in completion
order and balances GPSIMD scan time against dump-DMA bytes.  The host
picks the bf16 lane, rescales, applies sigmoid and unscatters; no mask,
no select, no segsum and no sigmoid run on device.
"""

import numpy as np
import ml_dtypes

BF = ml_dtypes.bfloat16
F8 = ml_dtypes.float8_e4m3fn

N_DRUGS = 4096
D = 512
N_CORES = 8
RSH, CSH = 2, 4           # row shards x col shards
RBLK = N_DRUGS // RSH     # 2048 rows of S per core
CBLK = N_DRUGS // CSH     # 1024 cols of S per core
NT = RBLK // 128          # 16 row-tile units per core
SU = 16.0                 # fp8 dynamic-range pre-scale
PKW = CBLK + RBLK         # packed matrix cols: zT8q | a8T

# logical units: (row_tile, col_lo, col_hi, kind)
# kind 'p' = pair gather + slot dump, 'd' = full bf16 block dump.
# Unit 0 split into halves pulls the first cast ~1us earlier.
LUNITS = tuple(
    (t, 0, 1024,
     "d" if (t % 2 == 1 and t < 13) or t == 14 else "p")
    for t in range(NT))
NL = len(LUNITS)
ACT_CAST = {0, 2, 4, 6, 8, 10, 12, 13, 15}  # ACT; rest DVE
N_WARM = 10                             # PE p-state warmup matmuls
# dump groups (logical 'p' units); DMA fires after the last one
GROUPS = ((0, 2), (4, 6), (8, 10), (12, 13), (15,))
# dma split points for the pack input (cols of [zT8q | a8T]);
# "ix" marks where the index-pack DMA is issued
DMA_SPLITS = ((0, 512), (CBLK, CBLK + 512), (512, CBLK), "ix",
              (CBLK + 512, CBLK + 1024), (CBLK + 1024, PKW))
MEMSET_ENG = "vector"                   # engine for warmup memsets

_cache = {}


def _lp():
    return tuple(i for i, u in enumerate(LUNITS) if u[3] == "p")


def _ld():
    return tuple(i for i, u in enumerate(LUNITS) if u[3] == "d")


def _offsets(nvs):
    """ix / out1 / out4 offsets per logical unit."""
    ixoff = np.zeros(NL + 1, np.int64)
    for i, (t, c0, c1, kd) in enumerate(LUNITS):
        ixoff[i + 1] = ixoff[i] + (nvs[i] // 16 if kd == "p" else 0)
    ooff = np.zeros(NL, np.int64)
    acc = 0
    for i in _lp():
        ooff[i] = acc
        acc += nvs[i]
    dacc = 0
    for i in _ld():
        ooff[i] = dacc
        dacc += LUNITS[i][2] - LUNITS[i][1]
    return ixoff, ooff, acc, dacc


def _build(nvs):
    import concourse.bass as bass  # noqa: F401
    import concourse.bacc as bacc
    import concourse.mybir as mybir
    import concourse.tile as tile

    f32 = mybir.dt.float32
    bf16 = mybir.dt.bfloat16
    fp8 = mybir.dt.float8e4
    u16 = mybir.dt.uint16
    u32 = mybir.dt.uint32
    DR = mybir.MatmulPerfMode.DoubleRow

    ixoff, ooff, pw, dw = _offsets(nvs)

    nc = bacc.Bacc("TRN2", target_bir_lowering=False, debug=False,
                   num_devices=N_CORES)

    PK = nc.dram_tensor("pack", [D, PKW], fp8, kind="ExternalInput")
    IX = nc.dram_tensor("ix", [128, int(ixoff[-1])], u16,
                        kind="ExternalInput")
    OUT1 = nc.dram_tensor("out1", [128, max(1, int(pw))], u32,
                          kind="ExternalOutput")
    OUT4 = nc.dram_tensor("out4", [128, max(1, int(dw))], u16,
                          kind="ExternalOutput")

    with tile.TileContext(nc) as tc:
        with (
            tc.tile_pool(name="big", bufs=1) as big,
            tc.tile_pool(name="sml", bufs=1) as sml,
            tc.tile_pool(name="psum", bufs=8, space="PSUM") as psum,
        ):
            pk_sb = big.tile([128, 4, PKW], fp8)
            pk_v = PK.ap().rearrange("(kc p) n -> p kc n", p=128)
            for sp in DMA_SPLITS:
                if sp == "ix":
                    xg_sb = sml.tile([128, int(ixoff[-1])], u16)
                    nc.sync.dma_start(xg_sb[:], IX.ap())
                else:
                    c0, c1 = sp
                    nc.sync.dma_start(pk_sb[:, :, c0:c1], pk_v[:, :, c0:c1])

            # PE p-state warmup during dma0: early anchor matmul pins
            # pe_busy_start, then a short burst bridges to the real work
            mseng = getattr(nc, MEMSET_ENG)
            wu_a = sml.tile([128, 2, 128], fp8)
            mseng.memset(wu_a[:], 0.0)
            wu = sml.tile([128, 2, 512], fp8)
            aps = psum.tile([128, 128], f32, tag="ps", bufs=4, name="wu_a")
            nc.tensor.matmul(aps[:], wu_a[:], wu_a[:],
                             start=True, stop=True, perf_mode=DR)
            mseng.memset(wu[:], 0.0)
            for i in range(N_WARM):
                wps = psum.tile([128, 512], f32, tag="ps", bufs=4,
                                name=f"wu_{i}")
                nc.tensor.matmul(wps[:], wu[:, :, 0:128], wu[:],
                                 start=True, stop=True, perf_mode=DR)

            # logical unit: DR matmuls -> psum -> bf16 cast -> gather or
            # full dump; 'p' slot tiles dump per GROUPS
            gtile = {}
            for li, (t, c0, c1, kd) in enumerate(LUNITS):
                cw = c1 - c0
                ps = psum.tile([128, cw], f32, tag="ps", bufs=4,
                               name=f"s_{li}")
                for nch in range(cw // 512):
                    for jc2 in range(2):
                        nc.tensor.matmul(
                            ps[:, nch * 512:(nch + 1) * 512],
                            pk_sb[:, 2 * jc2:2 * jc2 + 2,
                                  CBLK + t * 128:CBLK + (t + 1) * 128],
                            pk_sb[:, 2 * jc2:2 * jc2 + 2,
                                  c0 + nch * 512:c0 + (nch + 1) * 512],
                            start=(jc2 == 0), stop=(jc2 == 1), perf_mode=DR)
                sw = big.tile([128, cw], bf16, name=f"sw_{li}", tag="sw",
                              bufs=17)
                if li in ACT_CAST:
                    nc.scalar.copy(sw[:], ps[:])
                else:
                    nc.vector.tensor_copy(sw[:], ps[:])
                if kd == "d":
                    off = int(ooff[li])
                    nc.sync.dma_start(OUT4.ap()[:, off:off + cw],
                                      sw[:].bitcast(u16))
                    continue
                ixv = xg_sb[:, int(ixoff[li]):int(ixoff[li + 1])]
                grp = next(gr for gr in GROUPS if li in gr)
                gi = grp.index(li)
                gw = sum(nvs[u] for u in grp)
                if gi == 0:
                    gtile[grp] = big.tile([128, gw], f32, name=f"g_{li}")
                gt = gtile[grp]
                goff = sum(nvs[u] for u in grp[:gi])
                nc.gpsimd.indirect_copy(gt[:, goff:goff + nvs[li]],
                                        sw[:].bitcast(f32),
                                        ixv,
                                        i_know_ap_gather_is_preferred=True)
                if gi == len(grp) - 1:
                    off = int(ooff[grp[0]])
                    nc.sync.dma_start(
                        OUT1.ap()[:, off:off + gw],
                        gt[:].bitcast(u32))

    nc.compile()
    return nc


def _get_program(nvs):
    if nvs not in _cache:
        _cache[nvs] = _build(nvs)
    return _cache[nvs]


def kernel(z_drug, global_weight, local_diag, batch_edges, edge_sub_type_idx,
           **_unused):
    from concourse.bass_utils import run_bass_kernel_spmd

    z = np.asarray(z_drug, np.float32)
    W = np.asarray(global_weight, np.float32)
    ld = np.asarray(local_diag, np.float32)
    e = np.asarray(batch_edges)
    sub = int(np.asarray(edge_sub_type_idx))
    d = ld[sub]
    assert z.shape == (N_DRUGS, D) and W.shape == (D, D)
    B = e.shape[1]
    e0 = e[0].astype(np.int64)
    e1 = e[1].astype(np.int64)

    U = z * d                                           # [4096, 512] f32
    A = U @ W                                           # [4096, 512] f32
    zT8 = np.ascontiguousarray((U.T * SU)).astype(F8)   # [512, 4096] fp8
    a8T = np.ascontiguousarray((A.T * (SU * SU))).astype(F8)

    # per-row-tile logical unit lookup: [t, col>=512?] -> logical unit
    lu_of = np.zeros((NT, 2), np.int64)
    for i, (t, c0, c1, kd) in enumerate(LUNITS):
        if c0 == 0:
            lu_of[t, 0] = i
        if c1 == CBLK:
            lu_of[t, 1] = i
    lu_c0 = np.array([u[1] for u in LUNITS])
    is_d = np.array([u[3] == "d" for u in LUNITS])

    core = (e0 // RBLK) * CSH + e1 // CBLK
    rin = e0 % RBLK
    t = rin >> 7                                        # row tile
    g = (rin >> 4) & 7                                  # 16-partition group
    part = rin & 127
    cq = e1 % CBLK                                      # col within quarter
    lu = lu_of[t, (cq >= 512).astype(np.int64)]
    crel = cq - lu_c0[lu]                               # col within unit
    idx = crel >> 1                                     # f32 pair index

    # slot within each (core, lunit, group) bucket; nv padded per lunit
    order = np.lexsort((np.arange(B), g, lu, core))
    key = ((core * NL + lu) * 8 + g)[order]
    nb = N_CORES * NL * 8
    start = np.searchsorted(key, np.arange(nb), side="left")
    counts = np.bincount(key, minlength=nb)
    slot = np.arange(B) - start[key]
    cmax = counts.reshape(N_CORES, NL, 8).max(axis=(0, 2))
    # nv multiple of 32 keeps every ix slice 4-byte aligned (ISA mem4d)
    nvs = tuple(max(32, int(-(-int(c) // 32)) * 32) for c in cmax)
    ixoff, ooff, pw, dw = _offsets(nvs)

    cs = core[order]
    in_maps = []
    recs = []
    for c in range(N_CORES):
        m = order[cs == c]
        luc, gc = lu[m], g[m]
        ic = slot[cs == c]
        pm0 = ~is_d[luc]
        ix = np.zeros((128, int(ixoff[-1])), np.uint16)
        ix[16 * gc[pm0] + ic[pm0] % 16,
           ixoff[luc[pm0]] + ic[pm0] // 16] = idx[m][pm0].astype(np.uint16)
        ci, cj = divmod(c, CSH)
        pack = np.concatenate(
            [zT8[:, cj * CBLK:(cj + 1) * CBLK],
             a8T[:, ci * RBLK:(ci + 1) * RBLK]], axis=1)  # [512, PKW] fp8
        in_maps.append({"pack": pack, "ix": ix})
        recs.append((m, part[m], luc, ic, crel[m]))

    nc = _get_program(nvs)
    res = run_bass_kernel_spmd(nc, in_maps, list(range(N_CORES)))

    inv = 1.0 / (SU * SU * SU)
    out = np.empty(B, np.float32)
    for c in range(N_CORES):
        o1 = np.asarray(res.results[c]["out1"])         # [128, :] u32
        o4 = np.asarray(res.results[c]["out4"])         # [128, :] u16
        w1 = o1.view(np.uint16).reshape(128, -1, 2)     # bf16 lanes
        m, pt, luc, ic, crl = recs[c]
        dm = is_d[luc]
        pm = ~dm
        u16v = np.empty(len(m), np.uint16)
        u16v[dm] = o4[pt[dm], ooff[luc[dm]] + crl[dm]]
        u16v[pm] = w1[pt[pm], ooff[luc[pm]] + ic[pm], crl[pm] & 1]
        sc = (u16v.astype(np.uint32) << 16).view(np.float32) * inv
        out[m] = 1.0 / (1.0 + np.exp(-sc))
    return out


if __name__ == "__main__":
    dat = np.load("/root/problem/cached_io.npz")
    inputs = {k: dat[k] for k in ("z_drug", "global_weight", "local_diag",
                                  "batch_edges", "edge_sub_type_idx")}
    expected = dat["expected"]
    actual = kernel(**inputs)
    err = np.abs(actual - expected)
    print("max abs err:", err.max(), "mean:", err.mean())
    print("Relative error:", err.max() / np.abs(expected).max())


# revision 4
# speedup vs baseline: 1.0158x; 1.0012x over previous
"""Trainium2 Bass kernel for the Dedicom decoder problem — v5.

Math: with U = z * d, scores S = A @ U^T (A = U @ W host-precomputed).
Sharding 2 row-shards x 4 col-shards: core (i, j) owns S rows
[i*2048, (i+1)*2048) x cols [j*1024, (j+1)*1024), processed as logical
units (row-tile, col-range) computed with fp8 DoubleRow matmuls
(a8T x SU^2, zT8 x SU host-packed) and cast PSUM->SBUF bf16 (split
ACT/DVE).  'p' units: GPSIMD indirect_copy pulls each edge's bf16 PAIR
(f32-bitcast scan) into slot tiles that are dump-DMAd to DRAM.  'd'
units: the whole bf16 block is dump-DMAd (no gather; the host indexes
it directly).  Interleaving p/d keeps the SP DMA queue in completion
order and balances GPSIMD scan time against dump-DMA bytes; the last
unit is split into column halves cast on ACT and DVE in parallel.  The host
picks the bf16 lane, rescales, applies sigmoid and unscatters; no mask,
no select, no segsum and no sigmoid run on device.
"""

import numpy as np
import ml_dtypes

BF = ml_dtypes.bfloat16
F8 = ml_dtypes.float8_e4m3fn

N_DRUGS = 4096
D = 512
N_CORES = 8
RSH, CSH = 2, 4           # row shards x col shards
RBLK = N_DRUGS // RSH     # 2048 rows of S per core
CBLK = N_DRUGS // CSH     # 1024 cols of S per core
NT = RBLK // 128          # 16 row-tile units per core
SU = 16.0                 # fp8 dynamic-range pre-scale
PKW = CBLK + RBLK         # packed matrix cols: zT8q | a8T

# logical units: (row_tile, col_lo, col_hi, kind)
# kind 'p' = pair gather + slot dump, 'd' = full bf16 block dump.
LUNITS = tuple(
    (t, 0, 1024,
     "d" if (t % 2 == 1 and t < 13) or t == 14 else "p")
    for t in range(NT - 1)) + ((NT - 1, 0, 512, "p"),
                               (NT - 1, 512, 1024, "p"))
NL = len(LUNITS)
ACT_CAST = {0, 2, 4, 6, 8, 10, 12, 13, 15}  # ACT; rest DVE
# unit 15 is split: half 15 casts on ACT, half 16 on DVE, in
# parallel — balances the engines and shortens the tail chain
N_WARM = 10                             # PE p-state warmup matmuls
# dump groups (logical 'p' units); DMA fires after the last one
GROUPS = ((0, 2), (4, 6), (8, 10), (12, 13), (15, 16))
# dma split points for the pack input (cols of [zT8q | a8T]);
# "ix" marks where the index-pack DMA is issued
DMA_SPLITS = ((0, 512), (CBLK, CBLK + 512), (512, CBLK), "ix",
              (CBLK + 512, CBLK + 1024), (CBLK + 1024, PKW))
MEMSET_ENG = "vector"                   # engine for warmup memsets

_cache = {}


def _lp():
    return tuple(i for i, u in enumerate(LUNITS) if u[3] == "p")


def _ld():
    return tuple(i for i, u in enumerate(LUNITS) if u[3] == "d")


def _offsets(nvs):
    """ix / out1 / out4 offsets per logical unit."""
    ixoff = np.zeros(NL + 1, np.int64)
    for i, (t, c0, c1, kd) in enumerate(LUNITS):
        ixoff[i + 1] = ixoff[i] + (nvs[i] // 16 if kd == "p" else 0)
    ooff = np.zeros(NL, np.int64)
    acc = 0
    for i in _lp():
        ooff[i] = acc
        acc += nvs[i]
    dacc = 0
    for i in _ld():
        ooff[i] = dacc
        dacc += LUNITS[i][2] - LUNITS[i][1]
    return ixoff, ooff, acc, dacc


def _build(nvs):
    import concourse.bass as bass  # noqa: F401
    import concourse.bacc as bacc
    import concourse.mybir as mybir
    import concourse.tile as tile

    f32 = mybir.dt.float32
    bf16 = mybir.dt.bfloat16
    fp8 = mybir.dt.float8e4
    u16 = mybir.dt.uint16
    u32 = mybir.dt.uint32
    DR = mybir.MatmulPerfMode.DoubleRow

    ixoff, ooff, pw, dw = _offsets(nvs)

    nc = bacc.Bacc("TRN2", target_bir_lowering=False, debug=False,
                   num_devices=N_CORES)

    PK = nc.dram_tensor("pack", [D, PKW], fp8, kind="ExternalInput")
    IX = nc.dram_tensor("ix", [128, int(ixoff[-1])], u16,
                        kind="ExternalInput")
    OUT1 = nc.dram_tensor("out1", [128, max(1, int(pw))], u32,
                          kind="ExternalOutput")
    OUT4 = nc.dram_tensor("out4", [128, max(1, int(dw))], u16,
                          kind="ExternalOutput")

    with tile.TileContext(nc) as tc:
        with (
            tc.tile_pool(name="big", bufs=1) as big,
            tc.tile_pool(name="sml", bufs=1) as sml,
            tc.tile_pool(name="psum", bufs=8, space="PSUM") as psum,
        ):
            pk_sb = big.tile([128, 4, PKW], fp8)
            pk_v = PK.ap().rearrange("(kc p) n -> p kc n", p=128)
            for sp in DMA_SPLITS:
                if sp == "ix":
                    xg_sb = sml.tile([128, int(ixoff[-1])], u16)
                    nc.sync.dma_start(xg_sb[:], IX.ap())
                else:
                    c0, c1 = sp
                    nc.sync.dma_start(pk_sb[:, :, c0:c1], pk_v[:, :, c0:c1])

            # PE p-state warmup during dma0: early anchor matmul pins
            # pe_busy_start, then a short burst bridges to the real work
            mseng = getattr(nc, MEMSET_ENG)
            wu_a = sml.tile([128, 2, 128], fp8)
            mseng.memset(wu_a[:], 0.0)
            wu = sml.tile([128, 2, 512], fp8)
            aps = psum.tile([128, 128], f32, tag="ps", bufs=4, name="wu_a")
            nc.tensor.matmul(aps[:], wu_a[:], wu_a[:],
                             start=True, stop=True, perf_mode=DR)
            mseng.memset(wu[:], 0.0)
            for i in range(N_WARM):
                wps = psum.tile([128, 512], f32, tag="ps", bufs=4,
                                name=f"wu_{i}")
                nc.tensor.matmul(wps[:], wu[:, :, 0:128], wu[:],
                                 start=True, stop=True, perf_mode=DR)

            # logical unit: DR matmuls -> psum -> bf16 cast -> gather or
            # full dump; 'p' slot tiles dump per GROUPS
            gtile = {}
            for li, (t, c0, c1, kd) in enumerate(LUNITS):
                cw = c1 - c0
                ps = psum.tile([128, cw], f32, tag="ps", bufs=4,
                               name=f"s_{li}")
                for nch in range(cw // 512):
                    for jc2 in range(2):
                        nc.tensor.matmul(
                            ps[:, nch * 512:(nch + 1) * 512],
                            pk_sb[:, 2 * jc2:2 * jc2 + 2,
                                  CBLK + t * 128:CBLK + (t + 1) * 128],
                            pk_sb[:, 2 * jc2:2 * jc2 + 2,
                                  c0 + nch * 512:c0 + (nch + 1) * 512],
                            start=(jc2 == 0), stop=(jc2 == 1), perf_mode=DR)
                sw = big.tile([128, cw], bf16, name=f"sw_{li}", tag="sw",
                              bufs=17)
                if li in ACT_CAST:
                    nc.scalar.copy(sw[:], ps[:])
                else:
                    nc.vector.tensor_copy(sw[:], ps[:])
                if kd == "d":
                    off = int(ooff[li])
                    nc.sync.dma_start(OUT4.ap()[:, off:off + cw],
                                      sw[:].bitcast(u16))
                    continue
                ixv = xg_sb[:, int(ixoff[li]):int(ixoff[li + 1])]
                grp = next(gr for gr in GROUPS if li in gr)
                gi = grp.index(li)
                gw = sum(nvs[u] for u in grp)
                if gi == 0:
                    gtile[grp] = big.tile([128, gw], f32, name=f"g_{li}")
                gt = gtile[grp]
                goff = sum(nvs[u] for u in grp[:gi])
                nc.gpsimd.indirect_copy(gt[:, goff:goff + nvs[li]],
                                        sw[:].bitcast(f32),
                                        ixv,
                                        i_know_ap_gather_is_preferred=True)
                if gi == len(grp) - 1:
                    off = int(ooff[grp[0]])
                    nc.sync.dma_start(
                        OUT1.ap()[:, off:off + gw],
                        gt[:].bitcast(u32))

    nc.compile()
    return nc


def _get_program(nvs):
    if nvs not in _cache:
        _cache[nvs] = _build(nvs)
    return _cache[nvs]


def kernel(z_drug, global_weight, local_diag, batch_edges, edge_sub_type_idx,
           **_unused):
    from concourse.bass_utils import run_bass_kernel_spmd

    z = np.asarray(z_drug, np.float32)
    W = np.asarray(global_weight, np.float32)
    ld = np.asarray(local_diag, np.float32)
    e = np.asarray(batch_edges)
    sub = int(np.asarray(edge_sub_type_idx))
    d = ld[sub]
    assert z.shape == (N_DRUGS, D) and W.shape == (D, D)
    B = e.shape[1]
    e0 = e[0].astype(np.int64)
    e1 = e[1].astype(np.int64)

    U = z * d                                           # [4096, 512] f32
    A = U @ W                                           # [4096, 512] f32
    zT8 = np.ascontiguousarray((U.T * SU)).astype(F8)   # [512, 4096] fp8
    a8T = np.ascontiguousarray((A.T * (SU * SU))).astype(F8)

    # per-row-tile logical unit lookup: [t, col>=512?] -> logical unit
    lu_of = np.zeros((NT, 2), np.int64)
    for i, (t, c0, c1, kd) in enumerate(LUNITS):
        if c0 == 0:
            lu_of[t, 0] = i
        if c1 == CBLK:
            lu_of[t, 1] = i
    lu_c0 = np.array([u[1] for u in LUNITS])
    is_d = np.array([u[3] == "d" for u in LUNITS])

    core = (e0 // RBLK) * CSH + e1 // CBLK
    rin = e0 % RBLK
    t = rin >> 7                                        # row tile
    g = (rin >> 4) & 7                                  # 16-partition group
    part = rin & 127
    cq = e1 % CBLK                                      # col within quarter
    lu = lu_of[t, (cq >= 512).astype(np.int64)]
    crel = cq - lu_c0[lu]                               # col within unit
    idx = crel >> 1                                     # f32 pair index

    # slot within each (core, lunit, group) bucket; nv padded per lunit
    order = np.lexsort((np.arange(B), g, lu, core))
    key = ((core * NL + lu) * 8 + g)[order]
    nb = N_CORES * NL * 8
    start = np.searchsorted(key, np.arange(nb), side="left")
    counts = np.bincount(key, minlength=nb)
    slot = np.arange(B) - start[key]
    cmax = counts.reshape(N_CORES, NL, 8).max(axis=(0, 2))
    # nv multiple of 32 keeps every ix slice 4-byte aligned (ISA mem4d)
    nvs = tuple(max(32, int(-(-int(c) // 32)) * 32) for c in cmax)
    ixoff, ooff, pw, dw = _offsets(nvs)

    cs = core[order]
    in_maps = []
    recs = []
    for c in range(N_CORES):
        m = order[cs == c]
        luc, gc = lu[m], g[m]
        ic = slot[cs == c]
        pm0 = ~is_d[luc]
        ix = np.zeros((128, int(ixoff[-1])), np.uint16)
        ix[16 * gc[pm0] + ic[pm0] % 16,
           ixoff[luc[pm0]] + ic[pm0] // 16] = idx[m][pm0].astype(np.uint16)
        ci, cj = divmod(c, CSH)
        pack = np.concatenate(
            [zT8[:, cj * CBLK:(cj + 1) * CBLK],
             a8T[:, ci * RBLK:(ci + 1) * RBLK]], axis=1)  # [512, PKW] fp8
        in_maps.append({"pack": pack, "ix": ix})
        recs.append((m, part[m], luc, ic, crel[m]))

    nc = _get_program(nvs)
    res = run_bass_kernel_spmd(nc, in_maps, list(range(N_CORES)))

    inv = 1.0 / (SU * SU * SU)
    out = np.empty(B, np.float32)
    for c in range(N_CORES):
        o1 = np.asarray(res.results[c]["out1"])         # [128, :] u32
        o4 = np.asarray(res.results[c]["out4"])         # [128, :] u16
        w1 = o1.view(np.uint16).reshape(128, -1, 2)     # bf16 lanes
        m, pt, luc, ic, crl = recs[c]
        dm = is_d[luc]
        pm = ~dm
        u16v = np.empty(len(m), np.uint16)
        u16v[dm] = o4[pt[dm], ooff[luc[dm]] + crl[dm]]
        u16v[pm] = w1[pt[pm], ooff[luc[pm]] + ic[pm], crl[pm] & 1]
        sc = (u16v.astype(np.uint32) << 16).view(np.float32) * inv
        out[m] = 1.0 / (1.0 + np.exp(-sc))
    return out


if __name__ == "__main__":
    dat = np.load("/root/problem/cached_io.npz")
    inputs = {k: dat[k] for k in ("z_drug", "global_weight", "local_diag",
                                  "batch_edges", "edge_sub_type_idx")}
    expected = dat["expected"]
    actual = kernel(**inputs)
    err = np.abs(actual - expected)
    print("max abs err:", err.max(), "mean:", err.mean())
    print("Relative error:", err.max() / np.abs(expected).max())
